# revision 20
# baseline (speedup 1.0000x reference)
"""Trainium2 Bass kernel for the CRF mean-field layer (nn_CrfLayer).

Algorithm (C=2 classes, H=W=128, N=16384 pixels, 10 mean-field iterations):
  - softmax over 2 classes == sigmoid; sum-to-one lets us filter only class 0:
    sp1 complement via spatial norm, bl1 = bnorm - bl0.
  - bilateral kernel K[i,j] = exp(-0.5*d2) via one augmented dot product on
    the PE; operands are split-fp16 (hi+lo) so the moving operand streams at
    full rate: exponent = f_i.f_j - 0.5|f_i|^2 - 0.5|f_j|^2 + log s0_j, so
    exp(psum) = K[i,j]*s0[j] directly (log s0 folded into the matmul).
  - sharding: core c owns rows i in [c*2048, (c+1)*2048) of K (bl_i for its
    block).  j-range 3-way split per iteration:
      z = [0, N_PE): fp8e4 [j x i] tiles SBUF-RESIDENT (112KB/partition,
          built once in setup), consumed by the PE s0T-stationary matvec
          with zero per-iteration HBM traffic,
      y = [N_PE, N_PE+N_DVE): stored fp16 [i x j] tiles consumed by VectorE
          affine_mul_reduce against broadcast s0,
      x = rest: recomputed on the fly (PE exponent matmuls, 512-wide pairs —
          fp16 moving operands are ISA-capped at 512 — + ScalarE exp with
          accum_out; ln s0_j folded via Hs rows 19/20, written only for the
          x-range columns).
    The three paths are interleaved per i-tile slot so the PE fills the
    ScalarE pacing gaps (in-order engine queues + PSUM backpressure would
    otherwise serialize the z-matvecs behind the recompute matmuls).
  - all split-fp16 feature rows (21 x N) are precomputed on the HOST; setup
    only builds the stored K tiles + bnorm.  One 32KB AllGather per
    iteration shares the per-core bl0 shards; cheap per-pixel work
    (softmax, separable spatial filter, q update) is replicated.

split-fp16 contraction rows (k = 21), exponent = sum_k G[k,i] * Hs[k,j]:
   k0-4 : G fh_i    | Hs fh_j        k15: G msqh_i | Hs 1
   k5-9 : G fh_i    | Hs fl_j        k16: G msql_i | Hs 1
   k10-14: G fl_i   | Hs fh_j        k17: G 1      | Hs msqh_j
                                     k18: G 1      | Hs msql_j
                                     k19: G -1     | Hs nlsh_j   (-log s0 hi)
                                     k20: G -1     | Hs nlsl_j   (-log s0 lo)
"""

import sys
from contextlib import ExitStack

sys.path.insert(0, "/opt/trn_rl_repo")

import numpy as np

H = 128
W = 128
C = 2
N = H * W
M = 8
BLK = N // M  # 2048
TA, TB, TG = 160.0, 3.0, 3.0
ITERS = 10

# j-range 3-way split
N_PE = 7168
N_DVE = 4608
N_ACT = N - N_PE - N_DVE  # 4608
XOFF = N_PE + N_DVE  # x-range start
ICH = 512  # i-chunk width for the streamed matvec (one PSUM bank)
NG = BLK // ICH  # 4 i-groups per core
NPE_I = (N_PE // 128) * ICH  # 28672 fp8 bytes/partition per i-group
KCH = (N_ACT + 1023) // 1024  # recompute chunks per i-tile (last may be 512)
NBB = (N_DVE + N_ACT) // 1024  # setup build chunks per i-tile
KR = 21  # contraction rows

_CACHE = {}


def _gauss1d(n, theta):
    d = np.arange(n, dtype=np.float32)
    return np.exp(-0.5 * ((d[:, None] - d[None, :]) / theta) ** 2).astype(np.float32)


def _build(collective=True):
    import concourse.bass as bass
    import concourse.bacc as bacc
    from concourse import mybir, tile

    f32 = mybir.dt.float32
    f16 = mybir.dt.float16
    f8 = mybir.dt.float8e4
    AF = mybir.ActivationFunctionType
    ALU = mybir.AluOpType
    AX = mybir.AxisListType

    nc = bacc.Bacc("TRN2", target_bir_lowering=False, debug=False, num_devices=M)

    gs_d = nc.declare_dram_parameter("gs", [H, H], f32, isOutput=False)
    isn_d = nc.declare_dram_parameter("inv_sn", [H, W], f32, isOutput=False)
    ident_d = nc.declare_dram_parameter("ident", [128, 128], f32, isOutput=False)
    hs_d = nc.declare_dram_parameter("hs16", [KR, N], f16, isOutput=False)
    g_d = nc.declare_dram_parameter("g16", [KR, BLK], f16, isOutput=False)
    uin_d = nc.declare_dram_parameter("uin", [2, H, W], f32, isOutput=False)
    sw_d = nc.declare_dram_parameter("sw", [2, 2], f32, isOutput=False)
    bw_d = nc.declare_dram_parameter("bw", [2, 2], f32, isOutput=False)
    cm_d = nc.declare_dram_parameter("cm", [2, 2], f32, isOutput=False)
    cvec_d = nc.declare_dram_parameter("cvec", [1, 128], f32, isOutput=False)
    cv16_d = nc.declare_dram_parameter("cvec16", [1, 128], f16, isOutput=False)
    qout_d = nc.declare_dram_parameter("qout", [2, H, W], f32, isOutput=True)

    rg = [list(range(M))]

    with tile.TileContext(nc) as tc:
        with (
            tc.tile_pool(name="pers", bufs=1) as pers,
            tc.tile_pool(name="dramP", bufs=1, space="DRAM") as dramP,
            tc.tile_pool(name="dram_ag", bufs=2, space="DRAM") as dram_ag,
            tc.tile_pool(name="psmall", bufs=2, space="PSUM") as psmall,
        ):
            gs = pers.tile([H, H], f32)
            isn = pers.tile([H, W], f32)
            ident = pers.tile([128, 128], f32)
            # z-range K lives in SBUF for the whole kernel: [128 j-in-tile
            # partitions x (g, jt, i) free], 112KB/partition fp8.
            Kz = pers.tile([128, NG * NPE_I], f8)
            # iterations only touch the x-range columns of Hs (ACT path +
            # the per-iteration ln s0 rows 19/20); the full [KR, N] staging
            # lives in a setup-scoped pool.
            Hs16 = pers.tile([KR, N_ACT], f16)
            G16 = pers.tile([KR, BLK], f16)
            negc = pers.tile([128, 6], f32)
            U0m = pers.tile([H, W], f32)
            U1m = pers.tile([H, W], f32)
            q0 = pers.tile([H, W], f32)
            q1 = pers.tile([H, W], f32)
            inv_bn = pers.tile([H, W], f32)
            DU = pers.tile([H, W], f32)
            dAB = pers.tile([128, 2], f32)
            Dm = pers.tile([H, W], f32)
            ones16 = pers.tile([128, 1], f16)
            ones16r = pers.tile([1, 128], f16)

            K2_dram = dramP.tile([16, 128, N_DVE], f16)

            nc.sync.dma_start(gs[:], gs_d[:])
            nc.sync.dma_start(isn[:], isn_d[:])
            nc.sync.dma_start(ident[:], ident_d[:])
            nc.sync.dma_start(Hs16[:], hs_d[:, XOFF:N])
            nc.sync.dma_start(G16[:], g_d[:])
            nc.sync.dma_start(ones16[:], cv16_d[0:1, :].rearrange("a b -> b a"))
            nc.sync.dma_start(ones16r[:], cv16_d[0:1, :])

            # ---------------- setup ----------------
            with (
                tc.tile_pool(name="psb", bufs=2, space="PSUM") as psb,
                tc.tile_pool(name="psbn", bufs=2, space="PSUM") as psbn,
            ):
                _fs = ExitStack()
                ssb = _fs.enter_context(tc.tile_pool(name="ssb", bufs=1))

                # unaries and q init
                nc.sync.dma_start(U0m[:], uin_d[0])
                nc.sync.dma_start(U1m[:], uin_d[1])
                nc.vector.tensor_copy(q0[:], U0m[:])
                nc.vector.tensor_copy(q1[:], U1m[:])

                # coefficients: A = cm@(sw[:,0]-sw[:,1]), B = cm@(bw[:,0]-bw[:,1]),
                # Cc = cm@(sw[:,1]+bw[:,1]);  q_c = (U_c - Cc_c) - A_c*sp0 - B_c*bl0n
                swt = ssb.tile([2, 2], f32)
                bwt = ssb.tile([2, 2], f32)
                cmT = ssb.tile([2, 2], f32)
                nc.sync.dma_start(swt[:], sw_d[:])
                nc.sync.dma_start(bwt[:], bw_d[:])
                nc.sync.dma_start(cmT[:], cm_d[:, :].rearrange("a b -> b a"))
                m3 = ssb.tile([2, 3], f32)
                nc.vector.tensor_sub(m3[:, 0:1], swt[:, 0:1], swt[:, 1:2])
                nc.vector.tensor_sub(m3[:, 1:2], bwt[:, 0:1], bwt[:, 1:2])
                nc.vector.tensor_add(m3[:, 2:3], swt[:, 1:2], bwt[:, 1:2])
                ps_c = psmall.tile([2, 3], f32, tag="pss", name="ps_c")
                nc.tensor.matmul(ps_c[:], cmT[:], m3[:])
                c23 = ssb.tile([2, 3], f32)
                nc.vector.tensor_copy(c23[:], ps_c[:])
                cflat_d = dramP.tile([1, 6], f32)
                nc.sync.dma_start(cflat_d[:], c23[:])
                cflat = ssb.tile([1, 6], f32)
                nc.sync.dma_start(cflat[:], cflat_d[:])
                ones_r = ssb.tile([1, 128], f32)
                nc.sync.dma_start(ones_r[:], cvec_d[0:1, :])
                ps_b = psmall.tile([128, 6], f32, tag="pss", name="ps_b")
                nc.tensor.matmul(ps_b[:], ones_r[:], cflat[:])
                nc.vector.tensor_scalar_mul(negc[:], ps_b[:], -1.0)
                # D init is the RAW unary difference (q = U at t=0), before the
                # Cc adjustment below; the recurrence constant DU includes Cc.
                nc.vector.tensor_sub(Dm[:], U1m[:], U0m[:])
                nc.vector.tensor_scalar(U0m[:], U0m[:], negc[:, 2:3], None, op0=ALU.add)
                nc.vector.tensor_scalar(U1m[:], U1m[:], negc[:, 5:6], None, op0=ALU.add)
                # D-recurrence coefficients: D = DU + dA*sp0 + dB*bl0n
                nc.vector.tensor_sub(DU[:], U1m[:], U0m[:])
                nc.vector.tensor_sub(dAB[:, 0:1], negc[:, 3:4], negc[:, 0:1])
                nc.vector.tensor_sub(dAB[:, 1:2], negc[:, 4:5], negc[:, 1:2])

                _fs.close()
                _bs = ExitStack()
                k2p = _bs.enter_context(tc.tile_pool(name="k2p", bufs=1))
                ssb2 = _bs.enter_context(tc.tile_pool(name="ssb2", bufs=1))

                # full feature rows, setup-scoped (space reused by iter pools)
                Hs16f = ssb2.tile([KR, N], f16)
                nc.sync.dma_start(Hs16f[:], hs_d[:])

                # K block build + bnorm
                # phase A: [j x i] tiles for j in [0, N_PE) -> SBUF Kz.
                # Pure mm,mm->exp pipeline: the z bnorm accumulation runs as
                # standalone PE chains interleaved into phase B below, so the
                # PE never stalls waiting for an exp of its own output.
                bnflat = ssb2.tile([1, BLK], f32)
                for g in range(NG):
                    for pair in range(N_PE // 256):
                        jt = 2 * pair
                        ps = psb.tile([128, 1024], f32, tag="psb", name="ps")
                        nc.tensor.matmul(
                            ps[:, 0:512],
                            Hs16f[:, jt * 128:(jt + 1) * 128],
                            G16[:, g * 512:(g + 1) * 512],
                        )
                        nc.tensor.matmul(
                            ps[:, 512:1024],
                            Hs16f[:, (jt + 1) * 128:(jt + 2) * 128],
                            G16[:, g * 512:(g + 1) * 512],
                        )
                        base = g * NPE_I + pair * 1024
                        nc.scalar.activation(Kz[:, base:base + 1024], ps[:], AF.Exp)
                njt_s = N_PE // 128

                # phase B: [i x j] tiles for j in [N_PE, N) -> K2_dram (DVE range
                # only) + ScalarE-accumulated bnorm partials.  The z bnorm
                # PE chains (reading the phase-A Kz tiles) are interleaved
                # here so they fill PE slack while ScalarE drains phase B.
                bn_acc = ssb2.tile([128, 16 * NBB], f32)
                NDV_PAD = ((N_DVE + 1023) // 1024) * 1024
                for it in range(16):
                    if it < NG:
                        g = it
                        psg = psbn.tile([1, 512], f32, tag="psbn", name="psg")
                        for jt in range(njt_s):
                            base = g * NPE_I + jt * 512
                            nc.tensor.matmul(
                                psg[:], ones16[:], Kz[:, base:base + 512],
                                start=(jt == 0), stop=(jt == njt_s - 1),
                            )
                        nc.vector.tensor_copy(bnflat[:, g * 512:(g + 1) * 512], psg[:])
                    kstage2 = k2p.tile([128, NDV_PAD], f16, tag="kstage2", name="kstage2", bufs=2)
                    for b in range(NBB):
                        j0 = N_PE + b * 1024
                        ps2 = psb.tile([128, 1024], f32, tag="psb", name="ps2")
                        nc.tensor.matmul(
                            ps2[:, 0:512],
                            G16[:, it * 128:(it + 1) * 128],
                            Hs16f[:, j0:j0 + 512],
                        )
                        nc.tensor.matmul(
                            ps2[:, 512:1024],
                            G16[:, it * 128:(it + 1) * 128],
                            Hs16f[:, j0 + 512:j0 + 1024],
                        )
                        if b * 1024 < NDV_PAD:
                            aout = kstage2[:, b * 1024:(b + 1) * 1024]
                        else:
                            kb = k2p.tile([128, 1024], f16, tag="kb", name="kb", bufs=2)
                            aout = kb[:]
                        nc.scalar.activation(
                            aout, ps2[:], AF.Exp,
                            accum_out=bn_acc[:, it * NBB + b:it * NBB + b + 1],
                        )
                    for so in range((N_DVE + 1023) // 1024):
                        ws = min(1024, N_DVE - so * 1024)
                        nc.sync.dma_start(
                            K2_dram[it, :, so * 1024:so * 1024 + ws],
                            kstage2[:, so * 1024:so * 1024 + ws],
                        )
                bnact = ssb2.tile([128, 16], f32)
                nc.vector.reduce_sum(
                    bnact[:],
                    bn_acc[:, :].rearrange("p (a b) -> p a b", b=NBB),
                    axis=AX.X,
                )
                ps_bn = psmall.tile([16, 128], f32, tag="pss", name="ps_bn")
                nc.tensor.transpose(ps_bn[:], bnact[:], ident[:])
                tbn = ssb2.tile([16, 128], f32)
                nc.vector.tensor_copy(tbn[:], ps_bn[:])

                # AllGather bnorm (row0 = [i x j] part, row1 = PE part)
                agin0 = dram_ag.tile([2, BLK], f32, tag="agin", name="agin0")
                nc.sync.dma_start(
                    agin0[0:1, :].rearrange("o (a b) -> (o a) b", a=16), tbn[:]
                )
                nc.sync.dma_start(agin0[1:2, :], bnflat[:])
                if collective:
                    agout0 = dram_ag.tile(
                        [2 * M, BLK], f32, tag="agout", name="agout0",
                        addr_space="Shared",
                    )
                    nc.gpsimd.collective_compute(
                        "AllGather", ALU.bypass, replica_groups=rg,
                        ins=[agin0.opt()], outs=[agout0.opt()],
                    )
                else:
                    agout0 = dram_ag.tile([2 * M, BLK], f32, tag="agout", name="agout0")
                    nc.sync.dma_start(agout0[0:2, :], agin0[:])
                bnp = ssb2.tile([H, W], f32)
                bnp2 = ssb2.tile([H, W], f32)
                nc.sync.dma_start(
                    bnp[:],
                    agout0[:, :].rearrange("(r two) (a b) -> two r a b", two=2, a=16)[0],
                )
                nc.sync.dma_start(
                    bnp2[:],
                    agout0[:, :].rearrange("(r two) (a b) -> two r a b", two=2, a=16)[1],
                )
                nc.vector.tensor_add(bnp[:], bnp[:], bnp2[:])
                nc.vector.reciprocal(inv_bn[:], bnp[:])
                _bs.close()

            # ---------------- iterations ----------------
            with (
                tc.tile_pool(name="isb", bufs=2) as isb,
                tc.tile_pool(name="kdve", bufs=6) as kdve,
                tc.tile_pool(name="dacp", bufs=2) as dacp,
                tc.tile_pool(name="psact", bufs=2, space="PSUM") as psact,
                tc.tile_pool(name="psdma", bufs=2, space="PSUM") as psdma,
            ):
                for t in range(ITERS):
                    # softmax pieces: s0 = 1/(1+e^(q1-q0)), nls = log(1+e^(q1-q0))
                    E = isb.tile([H, W], f32, tag="E", name="E")
                    nc.scalar.activation(E[:], Dm[:], AF.Exp)
                    Uu = isb.tile([H, W], f32, tag="Uu", name="Uu")
                    nc.vector.tensor_scalar_add(Uu[:], E[:], 1.0)
                    S0 = isb.tile([H, W], f32, tag="S0", name="S0")
                    nc.vector.reciprocal(S0[:], Uu[:])
                    # ln s0 rows are only read by the ACT path for the x-range
                    # columns [N_PE+N_DVE, N) = image rows HX0..128.  Engines
                    # need a 0/32/64/96 base partition, so compute rows 64..128
                    # and round-trip only the needed tail rows.
                    HX0 = (N_PE + N_DVE) // W
                    NLS = isb.tile([64, W], f32, tag="NLS", name="NLS")
                    nc.scalar.activation(NLS[:], Uu[64:H, :], AF.Ln)
                    NLH = isb.tile([64, W], f16, tag="NLH", name="NLH")
                    nc.vector.tensor_copy(NLH[:], NLS[:])
                    NLHB = isb.tile([64, W], f32, tag="NLHB", name="NLHB")
                    nc.vector.tensor_copy(NLHB[:], NLH[:])
                    NLL32 = isb.tile([64, W], f32, tag="NLL32", name="NLL32")
                    nc.vector.tensor_sub(NLL32[:], NLS[:], NLHB[:])
                    NLL = isb.tile([64, W], f16, tag="NLL", name="NLL")
                    nc.vector.tensor_copy(NLL[:], NLL32[:])
                    nls_d = dramP.tile([2, N_ACT], f16, tag="nls_d", name="nls_d", bufs=2)
                    nc.sync.dma_start(nls_d[0:1, :], NLH[HX0 - 64:64, :])
                    nc.sync.dma_start(nls_d[1:2, :], NLL[HX0 - 64:64, :])
                    nc.sync.dma_start(Hs16[19:20, :], nls_d[0:1, :])
                    nc.sync.dma_start(Hs16[20:21, :], nls_d[1:2, :])

                    # s0 transposed (fp16) = streamed-matvec weights
                    ps_t = psmall.tile([128, 128], f32, tag="pss", name="ps_t")
                    nc.tensor.transpose(ps_t[:], S0[:], ident[:])
                    s0T = isb.tile([128, 128], f16, tag="s0T", name="s0T")
                    nc.vector.tensor_copy(s0T[:], ps_t[:])

                    # spatial: sp0 = (Gs @ S0 @ Gs) * inv_sn   (Gs symmetric)
                    ps1 = psmall.tile([128, 128], f32, tag="pss", name="ps1")
                    nc.tensor.matmul(ps1[:], gs[:], S0[:])
                    T1 = isb.tile([H, W], f32, tag="T1", name="T1")
                    nc.vector.tensor_copy(T1[:], ps1[:])
                    ps2 = psmall.tile([128, 128], f32, tag="pss", name="ps2")
                    nc.tensor.transpose(ps2[:], T1[:], ident[:])
                    T1t = isb.tile([H, W], f32, tag="T1t", name="T1t")
                    nc.vector.tensor_copy(T1t[:], ps2[:])
                    ps3 = psmall.tile([128, 128], f32, tag="pss", name="ps3")
                    nc.tensor.matmul(ps3[:], gs[:], T1t[:])
                    T2t = isb.tile([H, W], f32, tag="T2t", name="T2t")
                    nc.vector.tensor_copy(T2t[:], ps3[:])
                    ps4 = psmall.tile([128, 128], f32, tag="pss", name="ps4")
                    nc.tensor.transpose(ps4[:], T2t[:], ident[:])
                    SP0 = isb.tile([H, W], f32, tag="SP0", name="SP0")
                    nc.vector.tensor_mul(SP0[:], ps4[:], isn[:])

                    # z-path state: emit units early so the PE starts matvecs
                    # while the s0-broadcast round-trip is in flight.
                    blacc_dve = isb.tile([128, 16], f32, tag="blacc_dve", name="blacc_dve")
                    bldma = isb.tile([1, BLK], f32, tag="bldma", name="bldma", bufs=1)
                    acc = isb.tile([128, 16 * KCH], f32, tag="acc", name="acc")
                    njt = N_PE // 128
                    zunits = [(g, jt4) for g in range(NG) for jt4 in range(njt // 4)]
                    nzu = len(zunits)
                    zpos = 0
                    pd = None

                    def z_emit(nu):
                        nonlocal zpos, pd
                        for _ in range(nu):
                            if zpos >= nzu:
                                return
                            g, jt4 = zunits[zpos]
                            if jt4 == 0:
                                pd = psdma.tile([1, 512], f32, tag="pd", name="pd")
                            for jl in range(4):
                                jt = jt4 * 4 + jl
                                base = g * NPE_I + jt * 512
                                nc.tensor.matmul(
                                    pd[:],
                                    s0T[:, jt:jt + 1],
                                    Kz[:, base:base + 512],
                                    start=(jt == 0), stop=(jt == njt - 1),
                                )
                            if jt4 == njt // 4 - 1:
                                nc.vector.tensor_copy(
                                    bldma[:, g * 512:(g + 1) * 512], pd[:]
                                )
                            zpos += 1

                    # s0 broadcast across partitions (fp16) for the DVE range
                    # (v-range rows of the image only: rows 56..92)
                    HV0 = N_PE // W  # 56
                    HV1 = (N_PE + N_DVE) // W  # 92
                    # full-height fp16 convert (engine slices must be
                    # pow2-aligned); DMA picks out the 56..92 row slice.
                    S016 = isb.tile([H, W], f16, tag="S016", name="S016")
                    nc.vector.tensor_copy(S016[:], S0[:])
                    s0f_d = dramP.tile([1, N_DVE], f16, tag="s0f_d", name="s0f_d", bufs=2)
                    nc.sync.dma_start(
                        s0f_d[0:1, :].rearrange("o (a b) -> (o a) b", a=HV1 - HV0),
                        S016[HV0:HV1, :],
                    )
                    s0flat = isb.tile([1, N_DVE], f16, tag="s0flat", name="s0flat", bufs=1)
                    nc.sync.dma_start(s0flat[:], s0f_d[:])
                    s0bc = isb.tile([128, N_DVE], f16, tag="s0bc", name="s0bc", bufs=1)
                    for cb in range((N_DVE + 1023) // 1024):
                        wb = min(1024, N_DVE - cb * 1024)
                        psbc = psact.tile([128, 1024], f32, tag="pa", name="psbc")
                        j0b = cb * 1024
                        nc.tensor.matmul(
                            psbc[:, 0:512], ones16r[:], s0flat[0:1, j0b:j0b + 512],
                        )
                        if wb == 1024:
                            nc.tensor.matmul(
                                psbc[:, 512:1024], ones16r[:],
                                s0flat[0:1, j0b + 512:j0b + 1024],
                            )
                        nc.vector.tensor_copy(
                            s0bc[:, cb * 1024:cb * 1024 + wb], psbc[:, 0:wb]
                        )

                    # interleaved main loop over i-tile slots; z-units front-
                    # loaded: the PE is free early (ACT gated by the nls round-
                    # trip) and an early-empty z queue shortens the tail.
                    ZPROF = [7, 6, 5, 5, 4, 4, 3, 3, 3, 3, 3, 2, 2, 2, 2, 2]
                    for it in range(16):
                        z_emit(ZPROF[it])
                        for kk in range(KCH):
                            j0 = kk * 1024
                            wk = min(1024, N_ACT - j0)
                            pa = psact.tile([128, 1024], f32, tag="pa", name="pa")
                            nc.tensor.matmul(
                                pa[:, 0:512],
                                G16[:, it * 128:(it + 1) * 128],
                                Hs16[:, j0:j0 + 512],
                            )
                            if wk == 1024:
                                nc.tensor.matmul(
                                    pa[:, 512:1024],
                                    G16[:, it * 128:(it + 1) * 128],
                                    Hs16[:, j0 + 512:j0 + 1024],
                                )
                            nc.scalar.activation(
                                pa[:, 0:wk], pa[:, 0:wk], AF.Exp,
                                accum_out=acc[:, it * KCH + kk:it * KCH + kk + 1],
                            )
                        parts = []
                        for cb in range((N_DVE + 2047) // 2048):
                            sl0 = cb * 2048
                            wd = min(2048, N_DVE - sl0)
                            if wd == 2048:
                                kdv = kdve.tile([128, 2048], f16, tag="kdv", name="kdv", bufs=5)
                            else:
                                kdv = kdve.tile([128, wd], f16, tag="kdv1", name="kdv1", bufs=2)
                            nc.sync.dma_start(kdv[:], K2_dram[it, :, sl0:sl0 + wd])
                            scr = isb.tile([128, 2048], f16, tag="scr", name="scr", bufs=2)
                            aout = dacp.tile([128, 1], f32, tag="dacc", name="dacc", bufs=6)
                            nc.vector.affine_mul_reduce(
                                scr[:, 0:wd], aout[:], kdv[:],
                                s0bc[:, sl0:sl0 + wd], 1.0, 0.0,
                            )
                            parts.append(aout)
                        if len(parts) == 3:
                            t01 = dacp.tile([128, 1], f32, tag="dacc2", name="t01", bufs=2)
                            nc.vector.tensor_add(t01[:], parts[0][:], parts[1][:])
                            nc.vector.tensor_add(blacc_dve[:, it:it + 1], t01[:], parts[2][:])
                        else:
                            nc.vector.tensor_add(blacc_dve[:, it:it + 1], parts[0][:], parts[1][:])
                    z_emit(nzu)

                    blact = isb.tile([128, 16], f32, tag="blact", name="blact")
                    nc.vector.reduce_sum(
                        blact[:],
                        acc[:, :].rearrange("p (a b) -> p a b", b=KCH),
                        axis=AX.X,
                    )
                    nc.vector.tensor_add(blact[:], blact[:], blacc_dve[:])

                    # gather: row0 = act part (transposed to flat order), row1 = dma part
                    agin = dram_ag.tile([2, BLK], f32, tag="agin", name="agin")
                    nc.sync.dma_start(agin[1:2, :], bldma[:])
                    ps_bt = psmall.tile([16, 128], f32, tag="pss", name="ps_bt")
                    nc.tensor.transpose(ps_bt[:], blact[:], ident[:])
                    tbact = isb.tile([16, 128], f32, tag="tbact", name="tbact")
                    nc.vector.tensor_copy(tbact[:], ps_bt[:])
                    nc.sync.dma_start(
                        agin[0:1, :].rearrange("o (a b) -> (o a) b", a=16), tbact[:]
                    )
                    if collective:
                        agout = dram_ag.tile(
                            [2 * M, BLK], f32, tag="agout", name="agout",
                            addr_space="Shared",
                        )
                        nc.gpsimd.collective_compute(
                            "AllGather", ALU.bypass, replica_groups=rg,
                            ins=[agin.opt()], outs=[agout.opt()],
                        )
                    else:
                        agout = dram_ag.tile([2 * M, BLK], f32, tag="agout", name="agout")
                        nc.sync.dma_start(agout[0:2, :], agin[:])
                    APL = isb.tile([H, W], f32, tag="APL", name="APL")
                    DPL = isb.tile([H, W], f32, tag="DPL", name="DPL")
                    nc.sync.dma_start(
                        APL[:],
                        agout[:, :].rearrange("(r two) (a b) -> two r a b", two=2, a=16)[0],
                    )
                    nc.sync.dma_start(
                        DPL[:],
                        agout[:, :].rearrange("(r two) (a b) -> two r a b", two=2, a=16)[1],
                    )
                    BL = isb.tile([H, W], f32, tag="BL", name="BL")
                    nc.vector.tensor_add(BL[:], APL[:], DPL[:])
                    BLN = isb.tile([H, W], f32, tag="BLN", name="BLN")
                    nc.vector.tensor_mul(BLN[:], BL[:], inv_bn[:])

                    # D update: D = DU + dA*sp0 + dB*bl0n (q0/q1 materialized
                    # once after the loop from the final SP0/BLN)
                    if t < ITERS - 1:
                        tD = isb.tile([H, W], f32, tag="tD", name="tD")
                        nc.vector.scalar_tensor_tensor(
                            tD[:], SP0[:], dAB[:, 0:1], DU[:], op0=ALU.mult, op1=ALU.add
                        )
                        nc.vector.scalar_tensor_tensor(
                            Dm[:], BLN[:], dAB[:, 1:2], tD[:], op0=ALU.mult, op1=ALU.add
                        )
                    else:
                        t0 = isb.tile([H, W], f32, tag="t0", name="t0")
                        nc.vector.scalar_tensor_tensor(
                            t0[:], SP0[:], negc[:, 0:1], U0m[:], op0=ALU.mult, op1=ALU.add
                        )
                        nc.vector.scalar_tensor_tensor(
                            q0[:], BLN[:], negc[:, 1:2], t0[:], op0=ALU.mult, op1=ALU.add
                        )
                        t1 = isb.tile([H, W], f32, tag="t1", name="t1")
                        nc.vector.scalar_tensor_tensor(
                            t1[:], SP0[:], negc[:, 3:4], U1m[:], op0=ALU.mult, op1=ALU.add
                        )
                        nc.vector.scalar_tensor_tensor(
                            q1[:], BLN[:], negc[:, 4:5], t1[:], op0=ALU.mult, op1=ALU.add
                        )

            nc.sync.dma_start(qout_d[0], q0[:])
            nc.sync.dma_start(qout_d[1], q1[:])

    nc.compile()
    return nc


def _get_nc():
    if "nc" not in _CACHE:
        _CACHE["nc"] = _build()
    return _CACHE["nc"]


def _host_rows(F):
    """F: [5, n] f32 features -> [21, n] f16 rows (rows 19/20 zero)."""
    n = F.shape[1]
    fh = F.astype(np.float16)
    fl = (F - fh.astype(np.float32)).astype(np.float16)
    msq = (-0.5 * (F * F).sum(axis=0)).astype(np.float32)
    mh = msq.astype(np.float16)
    ml = (msq - mh.astype(np.float32)).astype(np.float16)
    out = np.zeros((KR, n), dtype=np.float16)
    return fh, fl, mh, ml, out


def kernel(**inputs):
    from concourse.bass_utils import run_bass_kernel_spmd

    unaries = np.ascontiguousarray(np.asarray(inputs["unaries"], dtype=np.float32))
    rgb = np.ascontiguousarray(np.asarray(inputs["rgb"], dtype=np.float32))
    sw = np.ascontiguousarray(np.asarray(inputs["spatial_ker_weights"], dtype=np.float32))
    bw = np.ascontiguousarray(np.asarray(inputs["bilateral_ker_weights"], dtype=np.float32))
    cm = np.ascontiguousarray(np.asarray(inputs["compatibility_matrix"], dtype=np.float32))

    gsm = _gauss1d(H, TG)
    rsum = gsm.sum(axis=1).astype(np.float32)
    inv_sn = (1.0 / np.outer(rsum, rsum)).astype(np.float32)
    ident = np.eye(128, dtype=np.float32)
    ys, xs = np.meshgrid(
        np.arange(H, dtype=np.float32), np.arange(W, dtype=np.float32), indexing="ij"
    )
    # features/3, mean-centered: rows = [3x/160, 3y/160, r, g, b]/3
    F = np.stack([
        (xs * (1.0 / TA)).reshape(N),
        (ys * (1.0 / TA)).reshape(N),
        rgb[0].reshape(N, 3).T[0] / 3.0,
        rgb[0].reshape(N, 3).T[1] / 3.0,
        rgb[0].reshape(N, 3).T[2] / 3.0,
    ]).astype(np.float32)
    F -= F.mean(axis=1, keepdims=True)

    fh, fl, mh, ml, hs16 = _host_rows(F)
    hs16[0:5] = fh
    hs16[5:10] = fl
    hs16[10:15] = fh
    hs16[15] = np.float16(1.0)
    hs16[16] = np.float16(1.0)
    hs16[17] = mh
    hs16[18] = ml

    uin = np.ascontiguousarray(unaries[0].transpose(2, 0, 1)).astype(np.float32)
    cvec = np.ones((1, 128), np.float32)
    cvec16 = np.ones((1, 128), np.float16)

    common = {
        "gs": gsm, "inv_sn": inv_sn, "ident": ident, "hs16": hs16,
        "uin": uin, "sw": sw, "bw": bw, "cm": cm, "cvec": cvec, "cvec16": cvec16,
    }
    in_maps = []
    for c in range(M):
        blk = slice(c * BLK, (c + 1) * BLK)
        Fb = F[:, blk]
        bfh = Fb.astype(np.float16)
        bfl = (Fb - bfh.astype(np.float32)).astype(np.float16)
        bmsq = (-0.5 * (Fb * Fb).sum(axis=0)).astype(np.float32)
        bmh = bmsq.astype(np.float16)
        bml = (bmsq - bmh.astype(np.float32)).astype(np.float16)
        g16 = np.zeros((KR, BLK), dtype=np.float16)
        g16[0:5] = bfh
        g16[5:10] = bfh
        g16[10:15] = bfl
        g16[15] = bmh
        g16[16] = bml
        g16[17] = np.float16(1.0)
        g16[18] = np.float16(1.0)
        g16[19] = np.float16(-1.0)
        g16[20] = np.float16(-1.0)
        m = dict(common)
        m["g16"] = g16
        in_maps.append(m)

    nc = _get_nc()
    import os

    trace = bool(int(os.environ.get("BASS_KERNEL_TRACE", "0")))
    res = run_bass_kernel_spmd(nc, in_maps, list(range(M)), trace=trace)
    _CACHE["exec_time_ns"] = res.exec_time_ns
    _CACHE["results"] = res
    q = np.asarray(res.results[0]["qout"])  # [2, H, W]
    return np.ascontiguousarray(q.transpose(1, 2, 0))[None]  # (1, H, W, 2)



# revision 25
# speedup vs baseline: 1.0402x; 1.0402x over previous
"""Trainium2 Bass kernel for the CRF mean-field layer (nn_CrfLayer).

Algorithm (C=2 classes, H=W=128, N=16384 pixels, 10 mean-field iterations):
  - softmax over 2 classes == sigmoid; sum-to-one lets us filter only class 0:
    sp1 complement via spatial norm, bl1 = bnorm - bl0.
  - bilateral kernel K[i,j] = exp(-0.5*d2) via one augmented dot product on
    the PE; operands are split-fp16 (hi+lo) so the moving operand streams at
    full rate: exponent = f_i.f_j - 0.5|f_i|^2 - 0.5|f_j|^2 + log s0_j, so
    exp(psum) = K[i,j]*s0[j] directly (log s0 folded into the matmul).
  - sharding: core c owns rows i in [c*2048, (c+1)*2048) of K (bl_i for its
    block).  j-range 3-way split per iteration:
      z = [0, N_PE): fp8e4 [j x i] tiles SBUF-RESIDENT (112KB/partition,
          built once in setup), consumed by the PE s0T-stationary matvec
          with zero per-iteration HBM traffic,
      y = [N_PE, N_PE+N_DVE): stored fp16 [i x j] tiles consumed by VectorE
          affine_mul_reduce against broadcast s0,
      x = rest: recomputed on the fly (PE exponent matmuls, 512-wide pairs —
          fp16 moving operands are ISA-capped at 512 — + ScalarE exp with
          accum_out; ln s0_j folded via Hs rows 19/20, written only for the
          x-range columns).
    The three paths are interleaved per i-tile slot so the PE fills the
    ScalarE pacing gaps (in-order engine queues + PSUM backpressure would
    otherwise serialize the z-matvecs behind the recompute matmuls).
  - all split-fp16 feature rows (21 x N) are precomputed on the HOST; setup
    only builds the stored K tiles + bnorm.  One 32KB AllGather per
    iteration shares the per-core bl0 shards; cheap per-pixel work
    (softmax, separable spatial filter, q update) is replicated.

split-fp16 contraction rows (k = 21), exponent = sum_k G[k,i] * Hs[k,j]:
   k0-4 : G fh_i    | Hs fh_j        k15: G msqh_i | Hs 1
   k5-9 : G fh_i    | Hs fl_j        k16: G msql_i | Hs 1
   k10-14: G fl_i   | Hs fh_j        k17: G 1      | Hs msqh_j
                                     k18: G 1      | Hs msql_j
                                     k19: G -1     | Hs nlsh_j   (-log s0 hi)
                                     k20: G -1     | Hs nlsl_j   (-log s0 lo)
"""

import sys
from contextlib import ExitStack

sys.path.insert(0, "/opt/trn_rl_repo")

import numpy as np

H = 128
W = 128
C = 2
N = H * W
M = 8
BLK = N // M  # 2048
TA, TB, TG = 160.0, 3.0, 3.0
ITERS = 10

# j-range 3-way split
N_PE = 7168
N_DVE = 4608
N_ACT = N - N_PE - N_DVE  # 4608
XOFF = N_PE + N_DVE  # x-range start
ICH = 512  # i-chunk width for the streamed matvec (one PSUM bank)
NG = BLK // ICH  # 4 i-groups per core
NPE_I = (N_PE // 128) * ICH  # 28672 fp8 bytes/partition per i-group
KCH = (N_ACT + 1023) // 1024  # recompute chunks per i-tile (last may be 512)
NBB = (N_DVE + N_ACT) // 1024  # setup build chunks per i-tile
KR = 21  # contraction rows

_CACHE = {}


def _gauss1d(n, theta):
    d = np.arange(n, dtype=np.float32)
    return np.exp(-0.5 * ((d[:, None] - d[None, :]) / theta) ** 2).astype(np.float32)


def _build(collective=True):
    import concourse.bass as bass
    import concourse.bacc as bacc
    from concourse import mybir, tile

    f32 = mybir.dt.float32
    f16 = mybir.dt.float16
    f8 = mybir.dt.float8e4
    AF = mybir.ActivationFunctionType
    ALU = mybir.AluOpType
    AX = mybir.AxisListType

    nc = bacc.Bacc("TRN2", target_bir_lowering=False, debug=False, num_devices=M)

    gs_d = nc.declare_dram_parameter("gs", [H, H], f32, isOutput=False)
    isn_d = nc.declare_dram_parameter("inv_sn", [H, W], f32, isOutput=False)
    ident_d = nc.declare_dram_parameter("ident", [128, 128], f32, isOutput=False)
    hs_d = nc.declare_dram_parameter("hs16", [KR, N], f16, isOutput=False)
    g_d = nc.declare_dram_parameter("g16", [KR, BLK], f16, isOutput=False)
    uin_d = nc.declare_dram_parameter("uin", [2, H, W], f32, isOutput=False)
    sw_d = nc.declare_dram_parameter("sw", [2, 2], f32, isOutput=False)
    bw_d = nc.declare_dram_parameter("bw", [2, 2], f32, isOutput=False)
    cm_d = nc.declare_dram_parameter("cm", [2, 2], f32, isOutput=False)
    cvec_d = nc.declare_dram_parameter("cvec", [1, 128], f32, isOutput=False)
    cv16_d = nc.declare_dram_parameter("cvec16", [1, 128], f16, isOutput=False)
    qout_d = nc.declare_dram_parameter("qout", [2, H, W], f32, isOutput=True)

    rg = [list(range(M))]

    with tile.TileContext(nc) as tc:
        with (
            tc.tile_pool(name="pers", bufs=1) as pers,
            tc.tile_pool(name="dramP", bufs=1, space="DRAM") as dramP,
            tc.tile_pool(name="dram_ag", bufs=2, space="DRAM") as dram_ag,
            tc.tile_pool(name="psmall", bufs=2, space="PSUM") as psmall,
        ):
            gs = pers.tile([H, H], f32)
            isn = pers.tile([H, W], f32)
            ident = pers.tile([128, 128], f32)
            # z-range K lives in SBUF for the whole kernel: [128 j-in-tile
            # partitions x (g, jt, i) free], 112KB/partition fp8.
            Kz = pers.tile([128, NG * NPE_I], f8)
            # iterations only touch the x-range columns of Hs (ACT path +
            # the per-iteration ln s0 rows 19/20); the full [KR, N] staging
            # lives in a setup-scoped pool.
            Hs16 = pers.tile([KR, N_ACT], f16)
            G16 = pers.tile([KR, BLK], f16)
            negc = pers.tile([128, 6], f32)
            U0m = pers.tile([H, W], f32)
            U1m = pers.tile([H, W], f32)
            q0 = pers.tile([H, W], f32)
            q1 = pers.tile([H, W], f32)
            inv_bn = pers.tile([H, W], f32)
            DU = pers.tile([H, W], f32)
            dAB = pers.tile([128, 2], f32)
            Dm = pers.tile([H, W], f32)
            ones16 = pers.tile([128, 1], f16)
            ones16r = pers.tile([1, 128], f16)

            K2_dram = dramP.tile([16, 128, N_DVE], f8)

            nc.sync.dma_start(gs[:], gs_d[:])
            nc.sync.dma_start(isn[:], isn_d[:])
            nc.sync.dma_start(ident[:], ident_d[:])
            nc.sync.dma_start(Hs16[:], hs_d[:, XOFF:N])
            nc.sync.dma_start(G16[:], g_d[:])
            nc.sync.dma_start(ones16[:], cv16_d[0:1, :].rearrange("a b -> b a"))
            nc.sync.dma_start(ones16r[:], cv16_d[0:1, :])

            # ---------------- setup ----------------
            with (
                tc.tile_pool(name="psb", bufs=2, space="PSUM") as psb,
                tc.tile_pool(name="psbn", bufs=2, space="PSUM") as psbn,
            ):
                _fs = ExitStack()
                ssb = _fs.enter_context(tc.tile_pool(name="ssb", bufs=1))

                # unaries and q init
                nc.sync.dma_start(U0m[:], uin_d[0])
                nc.sync.dma_start(U1m[:], uin_d[1])
                nc.vector.tensor_copy(q0[:], U0m[:])
                nc.vector.tensor_copy(q1[:], U1m[:])

                # coefficients: A = cm@(sw[:,0]-sw[:,1]), B = cm@(bw[:,0]-bw[:,1]),
                # Cc = cm@(sw[:,1]+bw[:,1]);  q_c = (U_c - Cc_c) - A_c*sp0 - B_c*bl0n
                swt = ssb.tile([2, 2], f32)
                bwt = ssb.tile([2, 2], f32)
                cmT = ssb.tile([2, 2], f32)
                nc.sync.dma_start(swt[:], sw_d[:])
                nc.sync.dma_start(bwt[:], bw_d[:])
                nc.sync.dma_start(cmT[:], cm_d[:, :].rearrange("a b -> b a"))
                m3 = ssb.tile([2, 3], f32)
                nc.vector.tensor_sub(m3[:, 0:1], swt[:, 0:1], swt[:, 1:2])
                nc.vector.tensor_sub(m3[:, 1:2], bwt[:, 0:1], bwt[:, 1:2])
                nc.vector.tensor_add(m3[:, 2:3], swt[:, 1:2], bwt[:, 1:2])
                ps_c = psmall.tile([2, 3], f32, tag="pss", name="ps_c")
                nc.tensor.matmul(ps_c[:], cmT[:], m3[:])
                c23 = ssb.tile([2, 3], f32)
                nc.vector.tensor_copy(c23[:], ps_c[:])
                cflat_d = dramP.tile([1, 6], f32)
                nc.sync.dma_start(cflat_d[:], c23[:])
                cflat = ssb.tile([1, 6], f32)
                nc.sync.dma_start(cflat[:], cflat_d[:])
                ones_r = ssb.tile([1, 128], f32)
                nc.sync.dma_start(ones_r[:], cvec_d[0:1, :])
                ps_b = psmall.tile([128, 6], f32, tag="pss", name="ps_b")
                nc.tensor.matmul(ps_b[:], ones_r[:], cflat[:])
                nc.vector.tensor_scalar_mul(negc[:], ps_b[:], -1.0)
                # D init is the RAW unary difference (q = U at t=0), before the
                # Cc adjustment below; the recurrence constant DU includes Cc.
                nc.vector.tensor_sub(Dm[:], U1m[:], U0m[:])
                nc.vector.tensor_scalar(U0m[:], U0m[:], negc[:, 2:3], None, op0=ALU.add)
                nc.vector.tensor_scalar(U1m[:], U1m[:], negc[:, 5:6], None, op0=ALU.add)
                # D-recurrence coefficients: D = DU + dA*sp0 + dB*bl0n
                nc.vector.tensor_sub(DU[:], U1m[:], U0m[:])
                nc.vector.tensor_sub(dAB[:, 0:1], negc[:, 3:4], negc[:, 0:1])
                nc.vector.tensor_sub(dAB[:, 1:2], negc[:, 4:5], negc[:, 1:2])

                _fs.close()
                _bs = ExitStack()
                k2p = _bs.enter_context(tc.tile_pool(name="k2p", bufs=1))
                ssb2 = _bs.enter_context(tc.tile_pool(name="ssb2", bufs=1))

                # full feature rows, setup-scoped (space reused by iter pools)
                Hs16f = ssb2.tile([KR, N], f16)
                nc.sync.dma_start(Hs16f[:], hs_d[:])

                # K block build + bnorm
                # phase A: [j x i] tiles for j in [0, N_PE) -> SBUF Kz.
                # Pure mm,mm->exp pipeline: the z bnorm accumulation runs as
                # standalone PE chains interleaved into phase B below, so the
                # PE never stalls waiting for an exp of its own output.
                bnflat = ssb2.tile([1, BLK], f32)
                for g in range(NG):
                    for pair in range(N_PE // 256):
                        jt = 2 * pair
                        ps = psb.tile([128, 1024], f32, tag="psb", name="ps")
                        nc.tensor.matmul(
                            ps[:, 0:512],
                            Hs16f[:, jt * 128:(jt + 1) * 128],
                            G16[:, g * 512:(g + 1) * 512],
                        )
                        nc.tensor.matmul(
                            ps[:, 512:1024],
                            Hs16f[:, (jt + 1) * 128:(jt + 2) * 128],
                            G16[:, g * 512:(g + 1) * 512],
                        )
                        base = g * NPE_I + pair * 1024
                        nc.scalar.activation(Kz[:, base:base + 1024], ps[:], AF.Exp)
                njt_s = N_PE // 128

                # phase B: [i x j] tiles for j in [N_PE, N) -> K2_dram (DVE range
                # only) + ScalarE-accumulated bnorm partials.  The z bnorm
                # PE chains (reading the phase-A Kz tiles) are interleaved
                # here so they fill PE slack while ScalarE drains phase B.
                bn_acc = ssb2.tile([128, 16 * NBB], f32)
                NDV_PAD = ((N_DVE + 1023) // 1024) * 1024
                for it in range(16):
                    if it < NG:
                        g = it
                        psg = psbn.tile([1, 512], f32, tag="psbn", name="psg")
                        for jt in range(njt_s):
                            base = g * NPE_I + jt * 512
                            nc.tensor.matmul(
                                psg[:], ones16[:], Kz[:, base:base + 512],
                                start=(jt == 0), stop=(jt == njt_s - 1),
                            )
                        nc.vector.tensor_copy(bnflat[:, g * 512:(g + 1) * 512], psg[:])
                    kstage2 = k2p.tile([128, NDV_PAD], f8, tag="kstage2", name="kstage2", bufs=2)
                    for b in range(NBB):
                        j0 = N_PE + b * 1024
                        ps2 = psb.tile([128, 1024], f32, tag="psb", name="ps2")
                        nc.tensor.matmul(
                            ps2[:, 0:512],
                            G16[:, it * 128:(it + 1) * 128],
                            Hs16f[:, j0:j0 + 512],
                        )
                        nc.tensor.matmul(
                            ps2[:, 512:1024],
                            G16[:, it * 128:(it + 1) * 128],
                            Hs16f[:, j0 + 512:j0 + 1024],
                        )
                        if b * 1024 < NDV_PAD:
                            aout = kstage2[:, b * 1024:(b + 1) * 1024]
                        else:
                            kb = k2p.tile([128, 1024], f8, tag="kb", name="kb", bufs=2)
                            aout = kb[:]
                        nc.scalar.activation(
                            aout, ps2[:], AF.Exp,
                            accum_out=bn_acc[:, it * NBB + b:it * NBB + b + 1],
                        )
                    for so in range((N_DVE + 1023) // 1024):
                        ws = min(1024, N_DVE - so * 1024)
                        nc.sync.dma_start(
                            K2_dram[it, :, so * 1024:so * 1024 + ws],
                            kstage2[:, so * 1024:so * 1024 + ws],
                        )
                bnact = ssb2.tile([128, 16], f32)
                nc.vector.reduce_sum(
                    bnact[:],
                    bn_acc[:, :].rearrange("p (a b) -> p a b", b=NBB),
                    axis=AX.X,
                )
                ps_bn = psmall.tile([16, 128], f32, tag="pss", name="ps_bn")
                nc.tensor.transpose(ps_bn[:], bnact[:], ident[:])
                tbn = ssb2.tile([16, 128], f32)
                nc.vector.tensor_copy(tbn[:], ps_bn[:])

                # AllGather bnorm (row0 = [i x j] part, row1 = PE part)
                agin0 = dram_ag.tile([2, BLK], f32, tag="agin", name="agin0")
                nc.sync.dma_start(
                    agin0[0:1, :].rearrange("o (a b) -> (o a) b", a=16), tbn[:]
                )
                nc.sync.dma_start(agin0[1:2, :], bnflat[:])
                if collective:
                    agout0 = dram_ag.tile(
                        [2 * M, BLK], f32, tag="agout", name="agout0",
                        addr_space="Shared",
                    )
                    nc.gpsimd.collective_compute(
                        "AllGather", ALU.bypass, replica_groups=rg,
                        ins=[agin0.opt()], outs=[agout0.opt()],
                    )
                else:
                    agout0 = dram_ag.tile([2 * M, BLK], f32, tag="agout", name="agout0")
                    nc.sync.dma_start(agout0[0:2, :], agin0[:])
                bnp = ssb2.tile([H, W], f32)
                bnp2 = ssb2.tile([H, W], f32)
                nc.sync.dma_start(
                    bnp[:],
                    agout0[:, :].rearrange("(r two) (a b) -> two r a b", two=2, a=16)[0],
                )
                nc.sync.dma_start(
                    bnp2[:],
                    agout0[:, :].rearrange("(r two) (a b) -> two r a b", two=2, a=16)[1],
                )
                nc.vector.tensor_add(bnp[:], bnp[:], bnp2[:])
                nc.vector.reciprocal(inv_bn[:], bnp[:])
                _bs.close()

            # ---------------- iterations ----------------
            with (
                tc.tile_pool(name="isb", bufs=2) as isb,
                tc.tile_pool(name="kdve", bufs=6) as kdve,
                tc.tile_pool(name="dacp", bufs=2) as dacp,
                tc.tile_pool(name="psact", bufs=2, space="PSUM") as psact,
                tc.tile_pool(name="psdma", bufs=2, space="PSUM") as psdma,
            ):
                for t in range(ITERS):
                    # softmax pieces: s0 = 1/(1+e^(q1-q0)), nls = log(1+e^(q1-q0))
                    E = isb.tile([H, W], f32, tag="E", name="E")
                    nc.scalar.activation(E[:], Dm[:], AF.Exp)
                    Uu = isb.tile([H, W], f32, tag="Uu", name="Uu")
                    nc.vector.tensor_scalar_add(Uu[:], E[:], 1.0)
                    S0 = isb.tile([H, W], f32, tag="S0", name="S0")
                    nc.vector.reciprocal(S0[:], Uu[:])
                    # ln s0 rows are only read by the ACT path for the x-range
                    # columns [N_PE+N_DVE, N) = image rows HX0..128.  Engines
                    # need a 0/32/64/96 base partition, so compute rows 64..128
                    # and round-trip only the needed tail rows.
                    HX0 = (N_PE + N_DVE) // W
                    NLS = isb.tile([64, W], f32, tag="NLS", name="NLS")
                    nc.scalar.activation(NLS[:], Uu[64:H, :], AF.Ln)
                    NLH = isb.tile([64, W], f16, tag="NLH", name="NLH")
                    nc.vector.tensor_copy(NLH[:], NLS[:])
                    NLHB = isb.tile([64, W], f32, tag="NLHB", name="NLHB")
                    nc.vector.tensor_copy(NLHB[:], NLH[:])
                    NLL32 = isb.tile([64, W], f32, tag="NLL32", name="NLL32")
                    nc.vector.tensor_sub(NLL32[:], NLS[:], NLHB[:])
                    NLL = isb.tile([64, W], f16, tag="NLL", name="NLL")
                    nc.vector.tensor_copy(NLL[:], NLL32[:])
                    nls_d = dramP.tile([2, N_ACT], f16, tag="nls_d", name="nls_d", bufs=2)
                    nc.sync.dma_start(nls_d[0:1, :], NLH[HX0 - 64:64, :])
                    nc.sync.dma_start(nls_d[1:2, :], NLL[HX0 - 64:64, :])
                    nc.sync.dma_start(Hs16[19:21, :], nls_d[0:2, :])

                    # s0 transposed (fp16) = streamed-matvec weights
                    ps_t = psmall.tile([128, 128], f32, tag="pss", name="ps_t")
                    nc.tensor.transpose(ps_t[:], S0[:], ident[:])
                    s0T = isb.tile([128, 128], f16, tag="s0T", name="s0T")
                    nc.vector.tensor_copy(s0T[:], ps_t[:])

                    # spatial: sp0 = (Gs @ S0 @ Gs) * inv_sn   (Gs symmetric)
                    ps1 = psmall.tile([128, 128], f32, tag="pss", name="ps1")
                    nc.tensor.matmul(ps1[:], gs[:], S0[:])
                    T1 = isb.tile([H, W], f32, tag="T1", name="T1")
                    nc.vector.tensor_copy(T1[:], ps1[:])
                    ps2 = psmall.tile([128, 128], f32, tag="pss", name="ps2")
                    nc.tensor.transpose(ps2[:], T1[:], ident[:])
                    T1t = isb.tile([H, W], f32, tag="T1t", name="T1t")
                    nc.vector.tensor_copy(T1t[:], ps2[:])
                    ps3 = psmall.tile([128, 128], f32, tag="pss", name="ps3")
                    nc.tensor.matmul(ps3[:], gs[:], T1t[:])
                    T2t = isb.tile([H, W], f32, tag="T2t", name="T2t")
                    nc.vector.tensor_copy(T2t[:], ps3[:])
                    ps4 = psmall.tile([128, 128], f32, tag="pss", name="ps4")
                    nc.tensor.transpose(ps4[:], T2t[:], ident[:])
                    SP0 = isb.tile([H, W], f32, tag="SP0", name="SP0")
                    nc.vector.tensor_mul(SP0[:], ps4[:], isn[:])

                    # z-path state: emit units early so the PE starts matvecs
                    # while the s0-broadcast round-trip is in flight.
                    blacc_dve = isb.tile([128, 16], f32, tag="blacc_dve", name="blacc_dve")
                    bldma = isb.tile([1, BLK], f32, tag="bldma", name="bldma", bufs=1)
                    acc = isb.tile([128, 16 * KCH], f32, tag="acc", name="acc")
                    njt = N_PE // 128
                    zunits = [(g, jt4) for g in range(NG) for jt4 in range(njt // 4)]
                    nzu = len(zunits)
                    zpos = 0
                    pd = None

                    def z_emit(nu):
                        nonlocal zpos, pd
                        for _ in range(nu):
                            if zpos >= nzu:
                                return
                            g, jt4 = zunits[zpos]
                            if jt4 == 0:
                                pd = psdma.tile([1, 512], f32, tag="pd", name="pd")
                            for jl in range(4):
                                jt = jt4 * 4 + jl
                                base = g * NPE_I + jt * 512
                                nc.tensor.matmul(
                                    pd[:],
                                    s0T[:, jt:jt + 1],
                                    Kz[:, base:base + 512],
                                    start=(jt == 0), stop=(jt == njt - 1),
                                )
                            if jt4 == njt // 4 - 1:
                                nc.vector.tensor_copy(
                                    bldma[:, g * 512:(g + 1) * 512], pd[:]
                                )
                            zpos += 1

                    # s0 broadcast across partitions (fp16) for the DVE range
                    # (v-range rows of the image only: rows 56..92)
                    HV0 = N_PE // W  # 56
                    HV1 = (N_PE + N_DVE) // W  # 92
                    # full-height fp16 convert (engine slices must be
                    # pow2-aligned); DMA picks out the 56..92 row slice.
                    S016 = isb.tile([H, W], f16, tag="S016", name="S016")
                    nc.vector.tensor_copy(S016[:], S0[:])
                    s0f_d = dramP.tile([1, N_DVE], f16, tag="s0f_d", name="s0f_d", bufs=2)
                    nc.sync.dma_start(
                        s0f_d[0:1, :].rearrange("o (a b) -> (o a) b", a=HV1 - HV0),
                        S016[HV0:HV1, :],
                    )
                    s0flat = isb.tile([1, N_DVE], f16, tag="s0flat", name="s0flat", bufs=1)
                    nc.sync.dma_start(s0flat[:], s0f_d[:])
                    s0bc = isb.tile([128, N_DVE], f16, tag="s0bc", name="s0bc", bufs=1)
                    for cb in range((N_DVE + 1023) // 1024):
                        wb = min(1024, N_DVE - cb * 1024)
                        psbc = psact.tile([128, 1024], f32, tag="pa", name="psbc")
                        j0b = cb * 1024
                        nc.tensor.matmul(
                            psbc[:, 0:512], ones16r[:], s0flat[0:1, j0b:j0b + 512],
                        )
                        if wb == 1024:
                            nc.tensor.matmul(
                                psbc[:, 512:1024], ones16r[:],
                                s0flat[0:1, j0b + 512:j0b + 1024],
                            )
                        nc.vector.tensor_copy(
                            s0bc[:, cb * 1024:cb * 1024 + wb], psbc[:, 0:wb]
                        )

                    # interleaved main loop over i-tile slots; z-units front-
                    # loaded: the PE is free early (ACT gated by the nls round-
                    # trip) and an early-empty z queue shortens the tail.
                    ZPROF = [10, 8, 7, 6, 5, 4, 3, 3, 2, 2, 2, 1, 1, 1, 1, 0]
                    for it in range(16):
                        z_emit(ZPROF[it])
                        for kk in range(KCH):
                            j0 = kk * 1024
                            wk = min(1024, N_ACT - j0)
                            pa = psact.tile([128, 1024], f32, tag="pa", name="pa")
                            nc.tensor.matmul(
                                pa[:, 0:512],
                                G16[:, it * 128:(it + 1) * 128],
                                Hs16[:, j0:j0 + 512],
                            )
                            if wk == 1024:
                                nc.tensor.matmul(
                                    pa[:, 512:1024],
                                    G16[:, it * 128:(it + 1) * 128],
                                    Hs16[:, j0 + 512:j0 + 1024],
                                )
                            nc.scalar.activation(
                                pa[:, 0:wk], pa[:, 0:wk], AF.Exp,
                                accum_out=acc[:, it * KCH + kk:it * KCH + kk + 1],
                            )
                        parts = []
                        for cb in range((N_DVE + 2047) // 2048):
                            sl0 = cb * 2048
                            wd = min(2048, N_DVE - sl0)
                            if wd == 2048:
                                kdv = kdve.tile([128, 2048], f8, tag="kdv", name="kdv", bufs=5)
                            else:
                                kdv = kdve.tile([128, wd], f8, tag="kdv1", name="kdv1", bufs=2)
                            nc.sync.dma_start(kdv[:], K2_dram[it, :, sl0:sl0 + wd])
                            scr = isb.tile([128, 2048], f16, tag="scr", name="scr", bufs=2)
                            aout = dacp.tile([128, 1], f32, tag="dacc", name="dacc", bufs=6)
                            nc.vector.affine_mul_reduce(
                                scr[:, 0:wd], aout[:], kdv[:],
                                s0bc[:, sl0:sl0 + wd], 1.0, 0.0,
                            )
                            parts.append(aout)
                        if len(parts) == 3:
                            t01 = dacp.tile([128, 1], f32, tag="dacc2", name="t01", bufs=2)
                            nc.vector.tensor_add(t01[:], parts[0][:], parts[1][:])
                            nc.vector.tensor_add(blacc_dve[:, it:it + 1], t01[:], parts[2][:])
                        else:
                            nc.vector.tensor_add(blacc_dve[:, it:it + 1], parts[0][:], parts[1][:])
                    z_emit(nzu)

                    blact = isb.tile([128, 16], f32, tag="blact", name="blact")
                    nc.vector.reduce_sum(
                        blact[:],
                        acc[:, :].rearrange("p (a b) -> p a b", b=KCH),
                        axis=AX.X,
                    )
                    nc.vector.tensor_add(blact[:], blact[:], blacc_dve[:])

                    # gather: row0 = act part (transposed to flat order), row1 = dma part
                    agin = dram_ag.tile([2, BLK], f32, tag="agin", name="agin")
                    nc.sync.dma_start(agin[1:2, :], bldma[:])
                    ps_bt = psmall.tile([16, 128], f32, tag="pss", name="ps_bt")
                    nc.tensor.transpose(ps_bt[:], blact[:], ident[:])
                    tbact = isb.tile([16, 128], f32, tag="tbact", name="tbact")
                    nc.vector.tensor_copy(tbact[:], ps_bt[:])
                    nc.sync.dma_start(
                        agin[0:1, :].rearrange("o (a b) -> (o a) b", a=16), tbact[:]
                    )
                    if collective:
                        agout = dram_ag.tile(
                            [2 * M, BLK], f32, tag="agout", name="agout",
                            addr_space="Shared",
                        )
                        nc.gpsimd.collective_compute(
                            "AllGather", ALU.bypass, replica_groups=rg,
                            ins=[agin.opt()], outs=[agout.opt()],
                        )
                    else:
                        agout = dram_ag.tile([2 * M, BLK], f32, tag="agout", name="agout")
                        nc.sync.dma_start(agout[0:2, :], agin[:])
                    APL = isb.tile([H, W], f32, tag="APL", name="APL")
                    DPL = isb.tile([H, W], f32, tag="DPL", name="DPL")
                    nc.sync.dma_start(
                        APL[:],
                        agout[:, :].rearrange("(r two) (a b) -> two r a b", two=2, a=16)[0],
                    )
                    nc.sync.dma_start(
                        DPL[:],
                        agout[:, :].rearrange("(r two) (a b) -> two r a b", two=2, a=16)[1],
                    )
                    BL = isb.tile([H, W], f32, tag="BL", name="BL")
                    nc.vector.tensor_add(BL[:], APL[:], DPL[:])
                    BLN = isb.tile([H, W], f32, tag="BLN", name="BLN")
                    nc.vector.tensor_mul(BLN[:], BL[:], inv_bn[:])

                    # D update: D = DU + dA*sp0 + dB*bl0n (q0/q1 materialized
                    # once after the loop from the final SP0/BLN)
                    if t < ITERS - 1:
                        tD = isb.tile([H, W], f32, tag="tD", name="tD")
                        nc.vector.scalar_tensor_tensor(
                            tD[:], SP0[:], dAB[:, 0:1], DU[:], op0=ALU.mult, op1=ALU.add
                        )
                        nc.vector.scalar_tensor_tensor(
                            Dm[:], BLN[:], dAB[:, 1:2], tD[:], op0=ALU.mult, op1=ALU.add
                        )
                    else:
                        t0 = isb.tile([H, W], f32, tag="t0", name="t0")
                        nc.vector.scalar_tensor_tensor(
                            t0[:], SP0[:], negc[:, 0:1], U0m[:], op0=ALU.mult, op1=ALU.add
                        )
                        nc.vector.scalar_tensor_tensor(
                            q0[:], BLN[:], negc[:, 1:2], t0[:], op0=ALU.mult, op1=ALU.add
                        )
                        t1 = isb.tile([H, W], f32, tag="t1", name="t1")
                        nc.vector.scalar_tensor_tensor(
                            t1[:], SP0[:], negc[:, 3:4], U1m[:], op0=ALU.mult, op1=ALU.add
                        )
                        nc.vector.scalar_tensor_tensor(
                            q1[:], BLN[:], negc[:, 4:5], t1[:], op0=ALU.mult, op1=ALU.add
                        )

            nc.sync.dma_start(qout_d[0], q0[:])
            nc.sync.dma_start(qout_d[1], q1[:])

    nc.compile()
    return nc


def _get_nc():
    if "nc" not in _CACHE:
        _CACHE["nc"] = _build()
    return _CACHE["nc"]


def _host_rows(F):
    """F: [5, n] f32 features -> [21, n] f16 rows (rows 19/20 zero)."""
    n = F.shape[1]
    fh = F.astype(np.float16)
    fl = (F - fh.astype(np.float32)).astype(np.float16)
    msq = (-0.5 * (F * F).sum(axis=0)).astype(np.float32)
    mh = msq.astype(np.float16)
    ml = (msq - mh.astype(np.float32)).astype(np.float16)
    out = np.zeros((KR, n), dtype=np.float16)
    return fh, fl, mh, ml, out


def kernel(**inputs):
    from concourse.bass_utils import run_bass_kernel_spmd

    unaries = np.ascontiguousarray(np.asarray(inputs["unaries"], dtype=np.float32))
    rgb = np.ascontiguousarray(np.asarray(inputs["rgb"], dtype=np.float32))
    sw = np.ascontiguousarray(np.asarray(inputs["spatial_ker_weights"], dtype=np.float32))
    bw = np.ascontiguousarray(np.asarray(inputs["bilateral_ker_weights"], dtype=np.float32))
    cm = np.ascontiguousarray(np.asarray(inputs["compatibility_matrix"], dtype=np.float32))

    gsm = _gauss1d(H, TG)
    rsum = gsm.sum(axis=1).astype(np.float32)
    inv_sn = (1.0 / np.outer(rsum, rsum)).astype(np.float32)
    ident = np.eye(128, dtype=np.float32)
    ys, xs = np.meshgrid(
        np.arange(H, dtype=np.float32), np.arange(W, dtype=np.float32), indexing="ij"
    )
    # features/3, mean-centered: rows = [3x/160, 3y/160, r, g, b]/3
    F = np.stack([
        (xs * (1.0 / TA)).reshape(N),
        (ys * (1.0 / TA)).reshape(N),
        rgb[0].reshape(N, 3).T[0] / 3.0,
        rgb[0].reshape(N, 3).T[1] / 3.0,
        rgb[0].reshape(N, 3).T[2] / 3.0,
    ]).astype(np.float32)
    F -= F.mean(axis=1, keepdims=True)

    fh, fl, mh, ml, hs16 = _host_rows(F)
    hs16[0:5] = fh
    hs16[5:10] = fl
    hs16[10:15] = fh
    hs16[15] = np.float16(1.0)
    hs16[16] = np.float16(1.0)
    hs16[17] = mh
    hs16[18] = ml

    uin = np.ascontiguousarray(unaries[0].transpose(2, 0, 1)).astype(np.float32)
    cvec = np.ones((1, 128), np.float32)
    cvec16 = np.ones((1, 128), np.float16)

    common = {
        "gs": gsm, "inv_sn": inv_sn, "ident": ident, "hs16": hs16,
        "uin": uin, "sw": sw, "bw": bw, "cm": cm, "cvec": cvec, "cvec16": cvec16,
    }
    in_maps = []
    for c in range(M):
        blk = slice(c * BLK, (c + 1) * BLK)
        Fb = F[:, blk]
        bfh = Fb.astype(np.float16)
        bfl = (Fb - bfh.astype(np.float32)).astype(np.float16)
        bmsq = (-0.5 * (Fb * Fb).sum(axis=0)).astype(np.float32)
        bmh = bmsq.astype(np.float16)
        bml = (bmsq - bmh.astype(np.float32)).astype(np.float16)
        g16 = np.zeros((KR, BLK), dtype=np.float16)
        g16[0:5] = bfh
        g16[5:10] = bfh
        g16[10:15] = bfl
        g16[15] = bmh
        g16[16] = bml
        g16[17] = np.float16(1.0)
        g16[18] = np.float16(1.0)
        g16[19] = np.float16(-1.0)
        g16[20] = np.float16(-1.0)
        m = dict(common)
        m["g16"] = g16
        in_maps.append(m)

    nc = _get_nc()
    import os

    trace = bool(int(os.environ.get("BASS_KERNEL_TRACE", "0")))
    res = run_bass_kernel_spmd(nc, in_maps, list(range(M)), trace=trace)
    _CACHE["exec_time_ns"] = res.exec_time_ns
    _CACHE["results"] = res
    q = np.asarray(res.results[0]["qout"])  # [2, H, W]
    return np.ascontiguousarray(q.transpose(1, 2, 0))[None]  # (1, H, W, 2)

